# revision 1
# baseline (speedup 1.0000x reference)
"""Trainium2 Bass kernel for nn_BPModel: LSTM encoder -> latent ODE (RK4) -> decoder.

Data-parallel over 8 NeuronCores: batch 4096 -> 512 per core. All parameters
replicated. Everything stays on-chip (SBUF) in a transposed [feature, batch]
layout; matmuls run as fp32r (1 col/cycle at the PE when K=128).

Measured HW constraint: matmuls with K<128 cost ~4x when mixed with K=128
matmuls, so every hot matmul is padded to K=128 with zero weight rows (zero
rows annihilate whatever shares the rhs tile).

LSTM (T=256 steps, 2 interleaved half-batch streams of 256 cols each):
  - gates PSUM layout per stream: [i | f | o | g] (256 cols each, 2 banks)
  - Whh contribution: 4 K=128 matmuls per stream
  - x-projection + bias folded into one K=128 matmul per gate chunk: the
    xt3 tile holds 16 timesteps' x rows (rows 2s, 2s+1) and a ones row
    (row 32); the Wball weight block has Wih at those rows, the summed
    bias at row 32, zeros elsewhere
  - one merged Sigmoid over [i|f|o], one Tanh over g, one Tanh over c
ODE: 9 fixed-grid Kutta-3/8 steps, 4 odef evals each; pn3 emits p rows at
  psum partitions 0-2, exp'd in place into a zeroed [128, B] rows tile,
  broadcast via K=128 matmuls against one-hot selector matrices.

Engine instructions carry a single HW sync-wait slot; a post-Tile pass
moves excess waits onto same-engine NoOps.
"""

import sys
import numpy as np

for _p in ("/opt/trn_rl_repo",):
    if _p not in sys.path:
        sys.path.insert(0, _p)

import concourse.bass as bass
import concourse.tile as tile
import concourse.mybir as mybir
import concourse.bass_utils as _bu
from concourse.bass_utils import run_bass_kernel_spmd


def _patched_bir_verify_and_optimise(tmpdir, inp="bir.json", outp="file.neff",
                                     arch=None, *, dve_root=None):
    """Same as bass_utils.bir_verify_and_optimise but with walrus LDW
    dedup enabled (redundant LDWEIGHTS elision for back-to-back matmuls
    sharing a stationary operand)."""
    cmd = [
        _bu.get_walrus_driver(),
        "--pass",
        ",".join(["birverifier", "runtime_memory_reservation", "lower_act",
                  "lower_dve", "lower_ap_offset", "codegen", "neff_packager"]),
        "-i", inp,
        "--neff-output-filename", outp,
        "--enable-birsim=true", "--mem-mode=physical", "--policy=0",
        "--enable-ldw-opt=true",
        "--assign-static-dmas-to-sp=false",
        "--dram-page-size=256", "--enable-neff-debug-info=true",
        "--jobs", "8",
        *_bu.get_walrus_args(
            _bu.get_bir_arch(tmpdir, inp) if arch is None else arch,
            tmpdir, dve_root=dve_root),
    ]
    result = _bu.run_command(cmd, cwd=tmpdir)
    if result is not None:
        from pathlib import Path
        (Path(tmpdir) / "log.txt").write_text(result.stdout)
    return f"{tmpdir}/{outp}"


_bu.bir_verify_and_optimise = _patched_bir_verify_and_optimise

F32 = mybir.dt.float32
F32R = mybir.dt.float32r
AF = mybir.ActivationFunctionType
ALU = mybir.AluOpType

NCORES = 8
B, T_FULL, D_IN, H, LAT = 4096, 256, 2, 128, 128
BP = B // NCORES          # 512 batch per core
BS = BP // 2              # 256 per stream
N_STEPS = 9
SXT = 16                  # t-slots per xt3 tile (x rows 0..31, ones at 32)

# gate order in PSUM regions: i, f, o, g  (pytorch packs i, f, g, o)
GATE_PERM = (0, 1, 3, 2)

# weight tensors (fp32r tiles); bias tensors (f32 tiles)
_W_SPECS = [
    ("Wball", [128, SXT * 512]),
    ("Whh", [128, 512]),
    ("erows", [128, 384]),
    ("fc1W", [128, 256]),
    ("fc2W", [128, 256]),
    ("pn1W", [128, 128]),
    ("pn2W", [128, 128]),
    ("pn3W", [128, 3]),
    ("cn1W", [128, 128]),
    ("cn2W", [128, 128]),
    ("cn3W", [128, 128]),
    ("dec1aW", [128, 128]),
    ("dec1b0W", [1, 128]), ("dec1b1W", [1, 128]), ("dec1b2W", [1, 128]),
    ("dec2W", [128, 128]),
    ("dec3W", [128, 2]),
]
_B_SPECS = [
    ("fc1b2", [128, 2]),
    ("fc2b", [128, 1]),
    ("pn1b", [128, 1]), ("pn2b", [128, 1]), ("pn3bias", [1, 5]),
    ("cn1b", [128, 1]), ("cn2b", [128, 1]), ("cn3b", [128, 1]),
    ("dec1b", [128, 1]),
    ("dec2b", [128, 1]),
    ("dec3b", [2, 1]),
]


def _f32(ap):
    return ap.bitcast(F32)


def _legalize_matmul_waits(nc):
    """Engine instructions carry a single HW sync-wait slot (walrus: 'Too
    many sync wait commands'). Move excess waits onto preceding NoOps on the
    same engine queue; engine FIFO order keeps correctness."""
    n_moved = 0
    for fn in nc.m.functions:
        for bb in fn.blocks:
            out = []
            for inst in bb.instructions:
                si = inst.sync_info
                if si is not None and si.on_wait and len(si.on_wait) > 1:
                    waits = list(si.on_wait)
                    for w in waits[:-1]:
                        nop = mybir.InstNoOp(
                            name=nc.get_next_instruction_name(),
                            engine=inst.engine,
                            ins=[], outs=[],
                            sync_info=mybir.SyncInfo(on_wait=[w], on_update=[]),
                        )
                        out.append(nop)
                    si.on_wait = waits[-1:]
                    n_moved += 1
                out.append(inst)
            bb.instructions[:] = out
    return n_moved


def build_program(T=T_FULL, n_steps=N_STEPS, use_f32r=True, debug=False,
                  legalize=True):
    RD = F32R if use_f32r else F32
    dt = 1.0 / n_steps
    nxt = (T + SXT - 1) // SXT
    nc = bass.Bass()
    ins = {}
    ins["xt3"] = nc.declare_dram_parameter("xt3", [128, nxt * BP], RD,
                                           isOutput=False)
    # aux constants: cols 0:128 ones, 128:384 zeros (initial h)
    ins["aux"] = nc.declare_dram_parameter("aux", [128, 128 + 2 * BS], RD,
                                           isOutput=False)
    for name, shape in _W_SPECS:
        ins[name] = nc.declare_dram_parameter(name, shape, RD, isOutput=False)
    for name, shape in _B_SPECS:
        ins[name] = nc.declare_dram_parameter(name, shape, F32, isOutput=False)
    y_out = nc.declare_dram_parameter("y", [2, BP], F32, isOutput=True)
    if debug:
        dbg_h = nc.declare_dram_parameter("dbg_h", [128, BP], F32, isOutput=True)
        dbg_z0 = nc.declare_dram_parameter("dbg_z0", [128, BP], F32, isOutput=True)
        dbg_zT = nc.declare_dram_parameter("dbg_zT", [128, BP], F32, isOutput=True)
        dbg_pr = nc.declare_dram_parameter("dbg_pr", [3, BP], F32, isOutput=True)
        dbg_k = nc.declare_dram_parameter("dbg_k", [128, 4 * BP], F32,
                                          isOutput=True)

    with tile.TileContext(nc) as tc:
        with (
            tc.tile_pool(name="const", bufs=1) as cp,
            tc.tile_pool(name="state", bufs=2) as st,
        ):
            sb = {}
            sb["xt3"] = cp.tile([128, nxt * BP], RD, tag="xt3", name="xt3")
            nc.sync.dma_start(sb["xt3"][:], ins["xt3"][:])
            for name, shape in _W_SPECS:
                sb[name] = cp.tile(shape, RD, tag=name, name=name)
                nc.sync.dma_start(sb[name][:], ins[name][:])
            for name, shape in _B_SPECS:
                sb[name] = cp.tile(shape, F32, tag=name, name=name)
                nc.sync.dma_start(sb[name][:], ins[name][:])
            paramA = cp.tile([1, BP], RD, tag="paramA")
            paramB = cp.tile([1, BP], RD, tag="paramB")
            paramC = cp.tile([1, BP], RD, tag="paramC")

            h = []
            c = []
            for s in range(2):
                ht = st.tile([128, BS], RD, tag=f"h{s}")
                ct = st.tile([128, BS], F32, tag=f"c{s}")
                nc.sync.dma_start(
                    ht[:], ins["aux"][:, 128 + BS * s : 128 + BS * (s + 1)])
                nc.gpsimd.memset(ct[:], 0.0)
                h.append(ht)
                c.append(ct)

            xt3 = sb["xt3"]
            Wball = sb["Wball"]
            Whh = sb["Whh"]

            # ------------------ LSTM ------------------
            # per-stream gates psum: one full bank per gate region so the
            # x-projection matmuls (no h dependency) can open all four
            # accumulation groups early and run during the elementwise chain
            with (
                tc.tile_pool(name="psA", bufs=1, space="PSUM") as gp,
                tc.tile_pool(name="work", bufs=3) as wp,
            ):
                for t in range(T):
                    til, slot = divmod(t, SXT)
                    gates = {}
                    sgm = {}
                    tg = {}
                    for s in range(2):
                        gates[s] = gp.tile([128, 2048], F32, tag=f"g{s}",
                                           name=f"g{s}_{t}")
                    for ci in range(4):
                        for s in range(2):
                            xsl = xt3[:, BP * til + BS * s
                                      : BP * til + BS * (s + 1)]
                            nc.tensor.matmul(
                                gates[s][:, 512 * ci : 512 * ci + BS],
                                Wball[:, 512 * slot + 128 * ci
                                      : 512 * slot + 128 * (ci + 1)],
                                xsl,
                                start=True, stop=False)
                    for ci in range(4):
                        for s in range(2):
                            nc.tensor.matmul(
                                gates[s][:, 512 * ci : 512 * ci + BS],
                                Whh[:, 128 * ci : 128 * (ci + 1)],
                                h[s][:],
                                start=False, stop=True)
                    for s in range(2):
                        sgm[s] = wp.tile([128, 3 * BS], F32, tag=f"sg{s}",
                                         name=f"sg{s}_{t}")
                        nc.scalar.activation(sgm[s][:, 0:BS], gates[s][:, 0:BS],
                                             AF.Sigmoid)
                        tg[s] = wp.tile([128, BS], F32, tag=f"tg{s}",
                                        name=f"tg{s}_{t}")
                        nc.scalar.activation(tg[s][:], gates[s][:, 1536 : 1536 + BS],
                                             AF.Tanh)
                        gfo = gates[s][:].rearrange(
                            "p (r q) -> p r q", r=4)[:, 1:3, 0:BS]
                        nc.scalar.activation(sgm[s][:, BS : 3 * BS], gfo, AF.Sigmoid)
                    cn = {}
                    for s in range(2):
                        t1 = wp.tile([128, BS], F32, tag=f"t1{s}", name=f"t1{s}_{t}")
                        nc.vector.tensor_tensor(
                            out=t1[:], in0=sgm[s][:, 0:BS], in1=tg[s][:], op=ALU.mult)
                        t2 = wp.tile([128, BS], F32, tag=f"t2{s}", name=f"t2{s}_{t}")
                        nc.vector.tensor_tensor(
                            out=t2[:], in0=sgm[s][:, BS : 2 * BS], in1=c[s][:],
                            op=ALU.mult)
                        cn[s] = st.tile([128, BS], F32, tag=f"c{s}", name=f"c{s}_{t}")
                        nc.vector.tensor_tensor(
                            out=cn[s][:], in0=t1[:], in1=t2[:], op=ALU.add)
                        c[s] = cn[s]
                    tct = {}
                    for s in range(2):
                        tct[s] = wp.tile([128, BS], F32, tag=f"tc{s}",
                                         name=f"tc{s}_{t}")
                        nc.scalar.activation(tct[s][:], cn[s][:], AF.Tanh)
                    for s in range(2):
                        hn_ = st.tile([128, BS], RD, tag=f"h{s}", name=f"h{s}_{t}")
                        nc.vector.tensor_tensor(
                            out=hn_[:], in0=sgm[s][:, 2 * BS : 3 * BS], in1=tct[s][:],
                            op=ALU.mult)
                        h[s] = hn_

            # ------------- encoder fc + ODE + decoder -------------
            with (
                tc.tile_pool(name="psB", bufs=7, space="PSUM") as pb,
                tc.tile_pool(name="ow", bufs=2) as ow,
            ):
                if debug:
                    for s in range(2):
                        nc.sync.dma_start(
                            dbg_h[:, BS * s : BS * (s + 1)], _f32(h[s][:]))
                # fc1: hN @ fc1W + b -> relu ; chunks j of the 256-dim output
                r1 = ow.tile([128, 1024], RD, tag="r1")
                for j in range(2):
                    pfc = pb.tile([128, 512], F32, tag="ps")
                    for s in range(2):
                        nc.tensor.matmul(
                            pfc[:, BS * s : BS * (s + 1)],
                            sb["fc1W"][:, 128 * j : 128 * (j + 1)],
                            h[s][:], start=True, stop=True)
                    nc.scalar.activation(
                        r1[:, 512 * j : 512 * (j + 1)], pfc[:], AF.Relu,
                        bias=sb["fc1b2"][:, j : j + 1])
                # fc2 (no relu)
                pz = pb.tile([128, BP], F32, tag="ps")
                nc.tensor.matmul(pz[:], sb["fc2W"][:, 0:128], r1[:, 0:512],
                                 start=True, stop=False)
                nc.tensor.matmul(pz[:], sb["fc2W"][:, 128:256], r1[:, 512:1024],
                                 start=False, stop=True)
                OSTR = 1
                OW = BP // OSTR
                zs = []
                for s_ in range(OSTR):
                    zt = ow.tile([128, OW], RD, tag=f"z{s_}")
                    nc.vector.tensor_scalar(
                        out=zt[:], in0=pz[:, OW * s_ : OW * (s_ + 1)],
                        scalar1=sb["fc2b"][:], scalar2=None, op0=ALU.add)
                    zs.append(zt)
                if debug:
                    for s_ in range(OSTR):
                        nc.sync.dma_start(dbg_z0[:, OW * s_ : OW * (s_ + 1)],
                                          _f32(zs[s_][:]))

                def mlp2(zin, W1, b1, W2, b2, tg1, tg2):
                    p1 = pb.tile([128, OW], F32, tag="ps")
                    nc.tensor.matmul(p1[:], sb[W1][:], zin[:],
                                     start=True, stop=True)
                    s1 = ow.tile([128, OW], RD, tag=tg1)
                    nc.scalar.activation(s1[:], p1[:], AF.Relu, bias=sb[b1][:])
                    p2 = pb.tile([128, OW], F32, tag="ps")
                    nc.tensor.matmul(p2[:], sb[W2][:], s1[:],
                                     start=True, stop=True)
                    s2 = ow.tile([128, OW], RD, tag=tg2)
                    nc.scalar.activation(s2[:], p2[:], AF.Relu, bias=sb[b2][:])
                    return s2

                def odef(zin, s_, first=False, ktag="k"):
                    sl = slice(OW * s_, OW * (s_ + 1))
                    # param + comp trunks interleaved stage-by-stage so both
                    # branches' matmuls pipeline on the PE FIFO
                    p1p = pb.tile([128, OW], F32, tag="ps", name="p1p")
                    nc.tensor.matmul(p1p[:], sb["pn1W"][:], zin[:],
                                     start=True, stop=True)
                    p1c = pb.tile([128, OW], F32, tag="ps", name="p1c")
                    nc.tensor.matmul(p1c[:], sb["cn1W"][:], zin[:],
                                     start=True, stop=True)
                    s1p = ow.tile([128, OW], RD, tag=f"sh1p{s_}")
                    nc.scalar.activation(s1p[:], p1p[:], AF.Relu, bias=sb["pn1b"][:])
                    s1c = ow.tile([128, OW], RD, tag=f"sh1c{s_}")
                    nc.scalar.activation(s1c[:], p1c[:], AF.Relu, bias=sb["cn1b"][:])
                    p2p = pb.tile([128, OW], F32, tag="ps", name="p2p")
                    nc.tensor.matmul(p2p[:], sb["pn2W"][:], s1p[:],
                                     start=True, stop=True)
                    p2c = pb.tile([128, OW], F32, tag="ps", name="p2c")
                    nc.tensor.matmul(p2c[:], sb["cn2W"][:], s1c[:],
                                     start=True, stop=True)
                    s2p = ow.tile([128, OW], RD, tag=f"sh2p{s_}")
                    nc.scalar.activation(s2p[:], p2p[:], AF.Relu, bias=sb["pn2b"][:])
                    s2c = ow.tile([128, OW], RD, tag=f"sh2c{s_}")
                    nc.scalar.activation(s2c[:], p2c[:], AF.Relu, bias=sb["cn2b"][:])
                    # pn3 col-split: three K=128 M=1 matmuls to base-0 psum rows
                    pp3 = [pb.tile([1, OW], F32, tag="ps", name=f"pp3_{r3}")
                           for r3 in range(3)]
                    for r3 in range(3):
                        nc.tensor.matmul(pp3[r3][:], sb["pn3W"][:, r3 : r3 + 1],
                                         s2p[:], start=True, stop=True)
                    rowA = ow.tile([1, OW], RD, tag=f"rowA{s_}")
                    rowB = ow.tile([1, OW], RD, tag=f"rowB{s_}")
                    rowC = ow.tile([1, OW], RD, tag=f"rowC{s_}")
                    pb3 = sb["pn3bias"]
                    # Rp = exp(p0+b0); Rd^-1 = exp(-(p1+b1)); C^-1 = exp(-(p2+b2))
                    nc.scalar.activation(rowA[:], pp3[0][:], AF.Exp,
                                         bias=pb3[0:1, 0:1], scale=1.0)
                    nc.scalar.activation(rowB[:], pp3[1][:], AF.Exp,
                                         bias=pb3[0:1, 1:2], scale=-1.0)
                    nc.scalar.activation(rowC[:], pp3[2][:], AF.Exp,
                                         bias=pb3[0:1, 2:3], scale=-1.0)
                    if first:
                        # params = exp(p + b) rows for the decoder
                        nc.vector.tensor_copy(out=paramA[0:1, sl], in_=rowA[:])
                        nc.scalar.activation(paramB[0:1, sl], pp3[1][:], AF.Exp,
                                             bias=pb3[0:1, 3:4], scale=1.0)
                        nc.scalar.activation(paramC[0:1, sl], pp3[2][:], AF.Exp,
                                             bias=pb3[0:1, 4:5], scale=1.0)
                    pcn = pb.tile([128, OW], F32, tag="ps")
                    nc.tensor.matmul(pcn[:], sb["cn3W"][:], s2c[:],
                                     start=True, stop=True)
                    # S_b = bcast(Rp) + bcast(Rd^-1); C_b = bcast(C^-1)
                    onesr = sb["erows"][0:1, 0:128]
                    Sb = pb.tile([128, OW], F32, tag="ps")
                    nc.tensor.matmul(Sb[:], onesr, rowA[:],
                                     start=True, stop=False)
                    nc.tensor.matmul(Sb[:], onesr, rowB[:],
                                     start=False, stop=True)
                    Cb = pb.tile([128, OW], F32, tag="ps")
                    nc.tensor.matmul(Cb[:], onesr, rowC[:],
                                     start=True, stop=True)
                    # k = (comp + cn3b - z*S_b) * C_b
                    d1 = ow.tile([128, OW], F32, tag=f"d1{s_}")
                    nc.vector.tensor_tensor(out=d1[:], in0=_f32(zin[:]), in1=Sb[:],
                                            op=ALU.mult)
                    d2 = ow.tile([128, OW], F32, tag=f"d2{s_}")
                    nc.vector.scalar_tensor_tensor(
                        out=d2[:], in0=pcn[:], scalar=sb["cn3b"][:], in1=d1[:],
                        op0=ALU.add, op1=ALU.subtract)
                    k = ow.tile([128, OW], F32, tag=ktag)
                    nc.vector.tensor_tensor(out=k[:], in0=d2[:], in1=Cb[:],
                                            op=ALU.mult)
                    return k

                def sttz(k_in0, scalar, ztile, tag):
                    # fp32r out: (k * scalar) + z
                    o = ow.tile([128, OW], RD, tag=tag)
                    nc.vector.scalar_tensor_tensor(
                        out=o[:], in0=k_in0[:], scalar=float(scalar),
                        in1=_f32(ztile[:]),
                        op0=ALU.mult, op1=ALU.add)
                    return o

                def sttk(in0, scalar, in1, tag):
                    # f32 out: (in0 * scalar) + in1
                    o = ow.tile([128, OW], F32, tag=tag)
                    nc.vector.scalar_tensor_tensor(
                        out=o[:], in0=in0[:], scalar=float(scalar), in1=in1[:],
                        op0=ALU.mult, op1=ALU.add)
                    return o

                def tt(in0, in1, op, tag):
                    o = ow.tile([128, OW], F32, tag=tag)
                    nc.vector.tensor_tensor(out=o[:], in0=in0[:], in1=in1[:], op=op)
                    return o

                for step in range(n_steps):
                    for s_ in range(OSTR):
                        z = zs[s_]
                        k1 = odef(z, s_, first=(step == 0), ktag=f"k1{s_}")
                        za = sttz(k1, dt / 3.0, z, f"za{s_}")   # z + dt/3 k1
                        k2 = odef(za, s_, ktag=f"k2{s_}")
                        if debug and step == 0 and s_ == 0:
                            nc.sync.dma_start(dbg_k[:, 0:OW], k1[:])
                            nc.sync.dma_start(dbg_k[:, BP : BP + OW], k2[:])
                            nc.sync.dma_start(dbg_k[:, 2 * BP : 2 * BP + OW],
                                              _f32(za[:]))
                        u1 = sttk(k1, -1.0 / 3.0, k2, f"u1{s_}")  # k2 - k1/3
                        zb = sttz(u1, dt, z, f"za{s_}")  # z + dt(k2 - k1/3)
                        k3 = odef(zb, s_, ktag=f"k3{s_}")
                        u2 = tt(k1, k2, ALU.subtract, f"u1{s_}")
                        u3 = tt(u2, k3, ALU.add, f"u2{s_}")
                        zc2 = sttz(u3, dt, z, f"za{s_}")  # z + dt(k1 - k2 + k3)
                        k4 = odef(zc2, s_, ktag=f"k4{s_}")
                        v1 = tt(k2, k3, ALU.add, f"u1{s_}")
                        v2 = sttk(v1, 3.0, k1, f"u2{s_}")  # k1 + 3(k2 + k3)
                        v3 = tt(v2, k4, ALU.add, f"u1{s_}")
                        zs[s_] = sttz(v3, dt / 8.0, z, f"z{s_}")  # z + dt/8 (...)

                for s_ in range(OSTR):
                    sl = slice(OW * s_, OW * (s_ + 1))
                    if debug:
                        nc.sync.dma_start(dbg_zT[:, sl], _f32(zs[s_][:]))
                        if s_ == 0:
                            nc.sync.dma_start(dbg_pr[0:1, :], _f32(paramA[:]))
                            nc.sync.dma_start(dbg_pr[1:2, :], _f32(paramB[:]))
                            nc.sync.dma_start(dbg_pr[2:3, :], _f32(paramC[:]))
                    # decoder: zc = [zT ; params]
                    pd1 = pb.tile([128, OW], F32, tag="ps")
                    nc.tensor.matmul(pd1[:], sb["dec1aW"][:], zs[s_][:],
                                     start=True, stop=False)
                    nc.tensor.matmul(pd1[:], sb["dec1b0W"][:], paramA[0:1, sl],
                                     start=False, stop=False)
                    nc.tensor.matmul(pd1[:], sb["dec1b1W"][:], paramB[0:1, sl],
                                     start=False, stop=False)
                    nc.tensor.matmul(pd1[:], sb["dec1b2W"][:], paramC[0:1, sl],
                                     start=False, stop=True)
                    sd1 = ow.tile([128, OW], RD, tag=f"sd1{s_}")
                    nc.scalar.activation(sd1[:], pd1[:], AF.Relu, bias=sb["dec1b"][:])
                    pd2 = pb.tile([128, OW], F32, tag="ps")
                    nc.tensor.matmul(pd2[:], sb["dec2W"][:], sd1[:],
                                     start=True, stop=True)
                    sd2 = ow.tile([128, OW], RD, tag=f"sd2{s_}")
                    nc.scalar.activation(sd2[:], pd2[:], AF.Relu, bias=sb["dec2b"][:])
                    pd3 = pb.tile([2, OW], F32, tag="ps")
                    nc.tensor.matmul(pd3[:], sb["dec3W"][:], sd2[:],
                                     start=True, stop=True)
                    yt = ow.tile([2, OW], F32, tag=f"y{s_}")
                    nc.vector.tensor_scalar(out=yt[:], in0=pd3[:],
                                            scalar1=sb["dec3b"][:],
                                            scalar2=None, op0=ALU.add)
                    nc.sync.dma_start(y_out[:, sl], yt[:])

    if legalize:
        _legalize_matmul_waits(nc)
    return nc


def prep_inputs(inputs, T=T_FULL):
    """Host-side marshaling: shard x, build xt3/Wball layouts, repack weights."""
    nxt = (T + SXT - 1) // SXT
    f = lambda a: np.ascontiguousarray(a, dtype=np.float32)
    x = f(inputs["x"])                      # [B, T, 2]
    Wih = f(inputs["lstm_Wih"])             # [2, 512]
    Whh = f(inputs["lstm_Whh"])             # [128, 512]
    bsum = f(inputs["lstm_bih"] + inputs["lstm_bhh"])   # [512]

    # permute gate chunks (i, f, g, o) -> (i, f, o, g)
    def permc(w):
        chunks = [w[..., 128 * cc : 128 * (cc + 1)] for cc in GATE_PERM]
        return np.concatenate(chunks, axis=-1)

    Wih_p, Whh_p, bsum_p = permc(Wih), permc(Whh), permc(bsum)

    # Wball: [128, SXT*512]; slot s: rows 2s,2s+1 = Wih rows, row 32 = bias
    Wball = np.zeros((128, SXT * 512), dtype=np.float32)
    for s in range(SXT):
        Wball[2 * s, 512 * s : 512 * (s + 1)] = Wih_p[0]
        Wball[2 * s + 1, 512 * s : 512 * (s + 1)] = Wih_p[1]
        Wball[32, 512 * s : 512 * (s + 1)] = bsum_p

    # xt3 per core: [128, nxt*BP]; tile t//SXT, x rows 2(t%SXT), ones row 32
    xt3_all = np.zeros((NCORES, 128, nxt * BP), dtype=np.float32)
    xs = x.reshape(NCORES, BP, T, 2)
    for core in range(NCORES):
        xc = xs[core]                       # [BP, T, 2]
        for t in range(T):
            til, slot = divmod(t, SXT)
            col0 = BP * til
            xt3_all[core, 2 * slot, col0 : col0 + BP] = xc[:, t, 0]
            xt3_all[core, 2 * slot + 1, col0 : col0 + BP] = xc[:, t, 1]
        xt3_all[core, 32, :] = 1.0

    # ones row (for ODE broadcasts via K=1 matmuls)
    erows = np.zeros((128, 384), dtype=np.float32)
    erows[0, 0:128] = 1.0

    def padw(w, rows, cols):
        out = np.zeros((rows, cols), dtype=np.float32)
        out[: w.shape[0], : w.shape[1]] = w
        return out

    def padb(b, rows):
        out = np.zeros((rows, 1), dtype=np.float32)
        out[: b.shape[0], 0] = b
        return out

    fc1_b = f(inputs["fc1_b"])
    fc2_W = f(inputs["fc2_W"])
    pn3_b = f(inputs["pn3_b"])
    # cols: b0, -b1, -b2, b1, b2
    pn3bias = np.array([[pn3_b[0], -pn3_b[1], -pn3_b[2], pn3_b[1], pn3_b[2]]],
                       dtype=np.float32)
    dec1_W = f(inputs["dec1_W"])            # [131, 128]

    common = {
        "Wball": Wball,
        "Whh": f(Whh_p),
        "erows": erows,
        "fc1W": f(inputs["fc1_W"]),
        "fc1b2": f(fc1_b.reshape(2, 128).T),
        "fc2W": f(np.concatenate([fc2_W[0:128], fc2_W[128:256]], axis=1)),
        "fc2b": f(inputs["fc2_b"][:, None]),
        "pn1W": padw(f(inputs["pn1_W"]), 128, 128),
        "pn1b": padb(f(inputs["pn1_b"]), 128),
        "pn2W": padw(f(inputs["pn2_W"]), 128, 128),
        "pn2b": f(inputs["pn2_b"][:, None]),
        "pn3W": f(inputs["pn3_W"]), "pn3bias": pn3bias,
        "cn1W": padw(f(inputs["cn1_W"]), 128, 128),
        "cn1b": padb(f(inputs["cn1_b"]), 128),
        "cn2W": padw(f(inputs["cn2_W"]), 128, 128),
        "cn2b": f(inputs["cn2_b"][:, None]),
        "cn3W": f(inputs["cn3_W"]), "cn3b": f(inputs["cn3_b"][:, None]),
        "dec1aW": f(dec1_W[0:128]),
        "dec1b0W": f(dec1_W[128:129]), "dec1b1W": f(dec1_W[129:130]),
        "dec1b2W": f(dec1_W[130:131]),
        "dec1b": f(inputs["dec1_b"][:, None]),
        "dec2W": padw(f(inputs["dec2_W"]), 128, 128),
        "dec2b": padb(f(inputs["dec2_b"]), 128),
        "dec3W": padw(f(inputs["dec3_W"]), 128, 2),
        "dec3b": f(inputs["dec3_b"][:, None]),
    }
    aux = np.zeros((128, 128 + 2 * BS), dtype=np.float32)
    aux[:, 0:128] = 1.0
    common["aux"] = aux

    in_maps = []
    for core in range(NCORES):
        m = dict(common)
        m["xt3"] = xt3_all[core]
        in_maps.append(m)
    return in_maps


_PROGRAM = None


def get_program():
    global _PROGRAM
    if _PROGRAM is None:
        _PROGRAM = build_program()
    return _PROGRAM


def run(inputs, **kwargs):
    nc = get_program()
    in_maps = prep_inputs(inputs)
    res = run_bass_kernel_spmd(nc, in_maps, list(range(NCORES)), **kwargs)
    outs = [res.results[i]["y"] for i in range(NCORES)]   # each [2, BP]
    y = np.concatenate([o.T for o in outs], axis=0).astype(np.float32)  # [B, 2]
    return y, res


def kernel(**inputs):
    y, _ = run(inputs)
    return y

